# revision 18
# baseline (speedup 1.0000x reference)
"""DiffMHA (differential multi-head attention) block on 8 TRN2 NeuronCores.

Problem: B=4, L=1024, D=1024, H=16 heads (DH=64). Three input streams
(e_v, e_a0, e_a1); Q/K projections per stream, scores summed across
streams, causal-masked softmax, context from the v-stream values,
out-projection + residual + LayerNorm.

Sharding: (batch, head-half) -> 8 cores. Core c handles batch c//2 and
heads (c%2)*8 .. +8: its 8 heads' Q/K/V projections (512 of 1024
channels), scores + softmax + context for all 1024 rows, and a partial
out-projection. A pairwise bf16 ReduceScatter sums the two partial
out-projections and hands each core its own 512 rows for residual +
LayerNorm. The RS runs in 2 row-chunks so the first chunk's comm fully
overlaps the second half's attention compute; the PE never waits on a
collective.

Key optimizations vs the previous version:
- Stream-stacked score contraction: Q/K for streams (v, a0) are packed
  into one 128-partition tile per head at projection time (partition-
  shifted PSUM->SBUF copies), so scores take 2 matmuls (K=128 + K=64)
  instead of 3 K=64 matmuls -> fewer LDWEIGHTS, fuller PE array.
- Causal trimming: score/ctx matmuls only cover k-tiles with k <= q
  (q in 256-row blocks); the q-range is trimmed per k-tile. The mask
  input is replaced by one [128,128] 0/1 diagonal template applied
  multiplicatively after exp (exp(-10000/8) == 0 in f32 exactly).
- Head-pair fused score tiles: both heads of a fold share one
  [128, 512] PSUM score tile and a single Exp activation.
- Softmax 1/sum via DVE reciprocal_approx_fast (~5x faster than
  nc.vector.reciprocal) once per fold (both heads at once).
- bf16 chunked ReduceScatter (1 MB in per chunk vs one 4 MB fp32 RS),
  first chunk overlapped with attention blocks 1/3.
- All host-side tensors are partition-major so every DMA line is
  contiguous (>=2KB descriptors instead of 256B).
- All biases applied as rank-1 ones-row matmuls into PSUM.
- Load order puts xt_v + wv first so the PE starts within ~5us and the
  HAM clock-gate warms once and stays warm.
"""

import os
import sys
import types

import ml_dtypes
import numpy as np

B, L, D, H = 4, 1024, 1024, 16
DH = D // H  # 64
HPC = H // 2  # heads per core
C = HPC * DH  # channels per core (512)
SCALE = float(1.0 / np.sqrt(DH))
EPS = 1e-12
NCORES = 8
BF16 = ml_dtypes.bfloat16

NDT = D // 128  # 8 contraction folds
NF = C // 128  # 4 channel folds per core (2 heads each)
NLT = L // 128  # 8 l-tiles
QBLK = 256
STREAMS = ("v", "a0", "a1")
RG = [[0, 1], [2, 3], [4, 5], [6, 7]]


def _install_ntff_hook():
    """Recreate antenv.axon_hooks (absent in this image) so
    run_bass_kernel_spmd(trace=True) can capture NTFF profiles."""
    if "antenv.axon_hooks" in sys.modules:
        return
    try:
        from trn_agent_boot.trn_boot import _ntff_profile_via_ctypes

        hook = _ntff_profile_via_ctypes("/opt/axon/libaxon_pjrt.so")
    except Exception:
        hook = None
    mod = types.ModuleType("antenv.axon_hooks")
    mod.get_axon_ntff_profile_hook = lambda: hook
    mod.set_axon_ntff_profile_hook = lambda h: None
    sys.modules["antenv.axon_hooks"] = mod


_install_ntff_hook()

import concourse.bass as bass  # noqa: E402,F401
import concourse.mybir as mybir  # noqa: E402
import concourse.tile as tile  # noqa: E402
from concourse import bacc  # noqa: E402
from concourse.bass_utils import run_bass_kernel_spmd  # noqa: E402

F32 = mybir.dt.float32
BF = mybir.dt.bfloat16
AF = mybir.ActivationFunctionType
ALU = mybir.AluOpType

_NC_CACHE = {}
LAST_RESULT = None


def build_nc():
    nc = bacc.Bacc("TRN2", target_bir_lowering=False, debug=False, num_devices=NCORES)

    # ---- DRAM parameters (per-core shards, host-prepped, partition-major) ----
    xt = {s: nc.declare_dram_parameter(f"xt_{s}", [128, NDT, L], BF, isOutput=False)
          for s in STREAMS}
    wq = {s: nc.declare_dram_parameter(f"wq_{s}", [NF, 128, NDT, 128], BF, isOutput=False)
          for s in STREAMS}
    wk = {s: nc.declare_dram_parameter(f"wk_{s}", [NF, 128, NDT, 128], BF, isOutput=False)
          for s in STREAMS}
    bqr = {s: nc.declare_dram_parameter(f"bq_{s}", [1, C], BF, isOutput=False)
           for s in STREAMS}
    bkr = {s: nc.declare_dram_parameter(f"bk_{s}", [1, C], BF, isOutput=False)
           for s in STREAMS}
    wv = nc.declare_dram_parameter("wv", [128, NDT, C], BF, isOutput=False)
    bv = nc.declare_dram_parameter("bv", [1, C], BF, isOutput=False)
    wout = nc.declare_dram_parameter("wout", [128, NF, D], BF, isOutput=False)
    bout_half = nc.declare_dram_parameter("bout_half", [1, D], BF, isOutput=False)
    tri = nc.declare_dram_parameter("tri", [128, 128], BF, isOutput=False)
    ev_res = nc.declare_dram_parameter("ev_res", [128, 4, D], F32, isOutput=False)
    gamma = nc.declare_dram_parameter("gamma", [1, D], F32, isOutput=False)
    beta = nc.declare_dram_parameter("beta", [1, D], F32, isOutput=False)
    out = nc.declare_dram_parameter("out", [L // 2, D], F32, isOutput=True)

    with tile.TileContext(nc) as tc:
        with (
            tc.tile_pool(name="persist", bufs=1) as persist,
            tc.tile_pool(name="xtp", bufs=2) as xtp,
            tc.tile_pool(name="wf", bufs=4) as wf,
            tc.tile_pool(name="attn", bufs=3) as attn_pool,
            tc.tile_pool(name="small", bufs=2) as small,
            tc.tile_pool(name="cpy", bufs=3) as cpy_pool,
            tc.tile_pool(name="ln", bufs=2) as ln_pool,
            tc.tile_pool(name="proj_ps", bufs=3, space="PSUM") as proj_ps,
            tc.tile_pool(name="sc_ps", bufs=3, space="PSUM") as sc_ps,
            tc.tile_pool(name="ctx_ps", bufs=2, space="PSUM") as ctx_ps,
            tc.tile_pool(name="dram", bufs=1, space="DRAM") as dram,
        ):
            # ---- persistent SBUF ----
            # cols 0:63 = V, cols 64:127 = ones -> the ctx matmul lands
            # the softmax sum on PSUM partitions 64..127 (64 copies), so
            # no partition_broadcast is needed for the normalization.
            vnat = persist.tile([128, NLT, HPC, 2 * DH], BF, tag="vnat")
            ctxt = persist.tile([128, NF, L], BF, tag="ctxt")
            ones_b = persist.tile([1, L], BF, tag="ones")
            eps_sb = persist.tile([128, 1], F32, tag="eps")
            tri_sb = persist.tile([128, 128], BF, tag="tri")
            gb_bc = persist.tile([128, 2, D], F32, tag="gbbc")
            wv_sb = persist.tile([128, NDT, C], BF, tag="wvsb")
            wout_sb = persist.tile([128, NF, D], BF, tag="woutsb")
            bv_sb = persist.tile([1, C], BF, tag="bvsb")
            bout_sb = persist.tile([1, D], BF, tag="boutsb")
            bqr_sb = {s: persist.tile([1, C], BF, tag=f"bqr{s}", name=f"bqr_{s}")
                      for s in STREAMS}
            bkr_sb = {s: persist.tile([1, C], BF, tag=f"bkr{s}", name=f"bkr_{s}")
                      for s in STREAMS}
            # stacked Q/K tiles: per head h, partitions 0-63 = stream v
            # head-h channels, 64-127 = stream a0; a1 keeps fold tiles.
            qs1 = [persist.tile([128, L], BF, tag=f"qs1_{h}", name=f"qs1_{h}")
                   for h in range(HPC)]
            ks1 = [persist.tile([128, L], BF, tag=f"ks1_{h}", name=f"ks1_{h}")
                   for h in range(HPC)]
            qt2 = [persist.tile([128, L], BF, tag=f"qt2_{f}", name=f"qt2_{f}")
                   for f in range(NF)]
            kt2 = [persist.tile([128, L], BF, tag=f"kt2_{f}", name=f"kt2_{f}")
                   for f in range(NF)]

            # ---- first loads: unblock the PE ASAP (per-dt split so the
            # first V-proj matmuls start after ~256KB, not 3MB) ----
            xt_sb = {}
            t = xtp.tile([128, NDT, L], BF, tag="xt", name="xt_v")
            for dt in range(NDT):
                nc.sync.dma_start(out=t[:, dt, :], in_=xt["v"][:, dt, :])
                nc.sync.dma_start(out=wv_sb[:, dt, :], in_=wv[:, dt, :])
            xt_sb["v"] = t
            nc.sync.dma_start(out=bv_sb[:, :], in_=bv[:, :])
            nc.vector.memset(ones_b[:, :], 1.0)
            nc.vector.memset(eps_sb[:, :], EPS)
            nc.vector.memset(vnat[:, :, :, DH:2 * DH], 1.0)

            # HAM warm-up: ~3.5us of dummy matmuls (no DMA dependency) while
            # the first input DMAs land, so real matmuls start at 2.4GHz.
            warm = proj_ps.tile([128, 512], F32, tag="proj")
            for i in range(16):
                nc.tensor.matmul(
                    warm[:, 0:256], ones_b[:, 0:128], ones_b[:, 0:256],
                    start=(i == 0), stop=(i == 15),
                )

            # ---- V projection (natural [l, c] layout + ones column) ----
            with nc.named_scope("vproj"):
                for lf in range(NLT):
                    ps = proj_ps.tile([128, 512], F32, tag="proj")
                    for dt in range(NDT):
                        nc.tensor.matmul(
                            ps[:, :],
                            xt_sb["v"][:, dt, lf * 128:(lf + 1) * 128],
                            wv_sb[:, dt, :],
                            start=(dt == 0), stop=False,
                        )
                    nc.tensor.matmul(
                        ps[:, :], ones_b[:, 0:128], bv_sb[:, :],
                        start=False, stop=True,
                    )
                    nc.scalar.copy(vnat[:, lf, :, 0:DH], ps[:, :])

            # ---- rest of the loads (overlap with V proj) ----
            t = xtp.tile([128, NDT, L], BF, tag="xt", name="xt_a0")
            nc.sync.dma_start(out=t[:, :, :], in_=xt["a0"][:, :, :])
            xt_sb["a0"] = t
            nc.sync.dma_start(out=tri_sb[:, :], in_=tri[:, :])
            for s in STREAMS:
                nc.sync.dma_start(out=bqr_sb[s][:, :], in_=bqr[s][:, :])
                nc.sync.dma_start(out=bkr_sb[s][:, :], in_=bkr[s][:, :])
            nc.sync.dma_start(out=bout_sb[:, :], in_=bout_half[:, :])
            nc.sync.dma_start(out=wout_sb[:, :, :], in_=wout[:, :, :])
            gsb = small.tile([1, D], F32, tag="gsb", bufs=1)
            bsb = small.tile([1, D], F32, tag="bsb", bufs=1)
            nc.sync.dma_start(out=gsb[:, :], in_=gamma[:, :])
            nc.sync.dma_start(out=bsb[:, :], in_=beta[:, :])
            nc.gpsimd.partition_broadcast(gb_bc[:, 0, :], gsb[:, :])
            nc.gpsimd.partition_broadcast(gb_bc[:, 1, :], bsb[:, :])

            # ---- Q/K projections ----
            # v/a0 results are split per head into the stacked qs1/ks1
            # tiles via partition-shifted copies; a1 keeps fold tiles.
            def qk_proj(s, f, which):
                w_dram = wq[s] if which == "q" else wk[s]
                brow = bqr_sb[s] if which == "q" else bkr_sb[s]
                dst1 = qs1 if which == "q" else ks1
                dst2 = qt2 if which == "q" else kt2
                w_t = wf.tile([128, NDT, 128], BF, tag="w", name=f"w{which}_{s}{f}")
                nc.sync.dma_start(out=w_t[:, :, :], in_=w_dram[f, :, :, :])
                for lh in range(2):
                    lsl = slice(lh * 512, (lh + 1) * 512)
                    ps = proj_ps.tile([128, 512], F32, tag="proj")
                    for dt in range(NDT):
                        nc.tensor.matmul(
                            ps[:, :], w_t[:, dt, :], xt_sb[s][:, dt, lsl],
                            start=(dt == 0), stop=False,
                        )
                    nc.tensor.matmul(
                        ps[:, :], brow[:, f * 128:(f + 1) * 128],
                        ones_b[:, 0:512], start=False, stop=True,
                    )
                    if s == "a1":
                        nc.scalar.copy(dst2[f][:, lsl], ps[:, :])
                    elif s == "v":
                        nc.scalar.copy(dst1[2 * f][0:64, lsl], ps[0:64, :])
                        nc.scalar.copy(dst1[2 * f + 1][0:64, lsl], ps[64:128, :])
                    else:  # a0
                        nc.scalar.copy(dst1[2 * f][64:128, lsl], ps[0:64, :])
                        nc.scalar.copy(dst1[2 * f + 1][64:128, lsl], ps[64:128, :])

            with nc.named_scope("qkproj_v_a0"):
                for s in ("v", "a0"):
                    for f in range(NF):
                        qk_proj(s, f, "q")
                        qk_proj(s, f, "k")

            # xt_a1 reuses xt_v's slot (xtp bufs=2); the DMA waits for the
            # last v-stream projection read automatically.
            t = xtp.tile([128, NDT, L], BF, tag="xt", name="xt_a1")
            nc.sync.dma_start(out=t[:, :, :], in_=xt["a1"][:, :, :])
            xt_sb["a1"] = t

            # ---- attention for one q-block of 256 rows ----
            def attn_block(b):
                q0 = b * QBLK
                for f in range(NF):
                    hA, hB = 2 * f, 2 * f + 1
                    cps = ctx_ps.tile([2 * DH, 2 * QBLK], F32, tag="ctx")
                    last_kt = 2 * b + 1

                    def ctx_mm(attn_sb, kt, trim, n):
                        nc.tensor.matmul(
                            cps[:, trim:QBLK], vnat[:, kt, hA, :],
                            attn_sb[:, 0:n],
                            start=(kt == 0), stop=False,
                        )
                        nc.tensor.matmul(
                            cps[:, QBLK + trim:2 * QBLK], vnat[:, kt, hB, :],
                            attn_sb[:, n:2 * n],
                            start=False, stop=(kt == last_kt),
                        )

                    # software pipeline: emit kt's ctx matmuls after kt+1's
                    # score matmuls, so the Exp latency never blocks the
                    # in-order PE queue.
                    pend = None
                    for kt in range(last_kt + 1):
                        trim = 128 if kt == last_kt else 0
                        n = QBLK - trim
                        qsl = slice(q0 + trim, q0 + QBLK)
                        ksl = slice(kt * 128, (kt + 1) * 128)
                        sps = sc_ps.tile([128, 2 * QBLK], F32, tag="sc")
                        # one accumulation group per PSUM bank: start only
                        # on the first matmul (clears has_written for the
                        # whole bank -> region B's first write stores).
                        # NOTE: keep [A1, A2, B1, B2] order — adjacent
                        # disjoint-row matmuls (A2/B2) would run CONCURRENTLY
                        # and drain into the same PSUM bank, which crashes
                        # the exec unit (NRT_EXEC_UNIT_UNRECOVERABLE).
                        for i, (r, h) in enumerate(
                            ((slice(0, n), hA), (slice(n, 2 * n), hB))
                        ):
                            p0 = (h % 2) * 64
                            nc.tensor.matmul(
                                sps[:, r], ks1[h][:, ksl], qs1[h][:, qsl],
                                start=(i == 0), stop=False,
                            )
                            nc.tensor.matmul(
                                sps[:, r], kt2[f][p0:p0 + 64, ksl],
                                qt2[f][p0:p0 + 64, qsl],
                                start=False, stop=(i == 1),
                            )
                        if pend is not None:
                            ctx_mm(*pend)
                        attn_sb = attn_pool.tile([128, 2 * QBLK], BF, tag="attn")
                        nc.scalar.activation(
                            attn_sb[:, 0:2 * n], sps[:, 0:2 * n], AF.Exp, scale=SCALE
                        )
                        if kt >= 2 * b:  # diagonal k-tile: 0/1 causal mask
                            nc.vector.tensor_mul(
                                attn_sb[:, 0:128], attn_sb[:, 0:128], tri_sb[:, :]
                            )
                            nc.vector.tensor_mul(
                                attn_sb[:, n:n + 128], attn_sb[:, n:n + 128],
                                tri_sb[:, :],
                            )
                        pend = (attn_sb, kt, trim, n)
                    ctx_mm(*pend)
                    # normalize both heads: ctx /= sum. PSUM rows 64:128
                    # hold 64 copies of the sum (ones-columns trick). Copy
                    # to SBUF first: custom-DVE ops mis-execute on HW when
                    # reading PSUM directly.
                    ssb = small.tile([64, 2 * QBLK], F32, tag="ssb")
                    nc.vector.tensor_copy(ssb[:, :], cps[DH:2 * DH, :])
                    invbc = small.tile([64, 2 * QBLK], F32, tag="invbc")
                    nc.vector.reciprocal_approx_fast(invbc[:, :], ssb[:, :])
                    nc.vector.tensor_mul(
                        ctxt[0:64, f, q0:q0 + QBLK],
                        cps[0:DH, 0:QBLK], invbc[:, 0:QBLK],
                    )
                    nc.vector.tensor_mul(
                        ctxt[64:128, f, q0:q0 + QBLK],
                        cps[0:DH, QBLK:2 * QBLK], invbc[:, QBLK:2 * QBLK],
                    )

            # a1 projections fold-major, interleaved with attention on the
            # first q-block so the PE pipeline never drains.
            with nc.named_scope("qkproj_a1_attn0"):
                for f in range(NF):
                    qk_proj("a1", f, "q")
                    qk_proj("a1", f, "k")
                attn_block(0)
            with nc.named_scope("attn2"):
                attn_block(2)

            # ---- partial out-projection + chunked bf16 ReduceScatter ----
            # Chunk ck covers global rows {j*512 + ck*256 ..+256, j=0,1};
            # the RS hands each core the summed rows it owns (rank order
            # == chunk order makes this core-uniform).
            rs_in = [dram.tile([2, 128, D], BF, name=f"rs_in{k}") for k in range(4)]
            rs_out = [dram.tile([128, D], BF, name=f"rs_out{k}") for k in range(4)]

            def out_partial(ck):
                with nc.named_scope(f"outpart{ck}"):
                    for j in range(2):
                        gq0 = j * 512 + ck * 128
                        for half in range(2):
                            dsl = slice(half * 512, (half + 1) * 512)
                            ps = proj_ps.tile([128, 512], F32, tag="proj")
                            for cf in range(NF):
                                nc.tensor.matmul(
                                    ps[:, :], ctxt[:, cf, gq0:gq0 + 128],
                                    wout_sb[:, cf, dsl],
                                    start=(cf == 0), stop=False,
                                )
                            nc.tensor.matmul(
                                ps[:, :], ones_b[:, 0:128], bout_sb[:, dsl],
                                start=False, stop=True,
                            )
                            pt = cpy_pool.tile([128, 512], BF, tag="pt")
                            nc.vector.tensor_copy(pt[:, :], ps[:, :])
                            nc.sync.dma_start(
                                out=rs_in[ck][j, :, dsl], in_=pt[:, :],
                            )
                    nc.gpsimd.collective_compute(
                        "ReduceScatter", ALU.add, replica_groups=RG,
                        ins=[rs_in[ck].opt()], outs=[rs_out[ck].opt()],
                    )

            # block 1 before the first out_partials: the in-order PE queue
            # must not park out-proj matmuls (waiting on block 2's
            # normalization) ahead of ready attention work.
            with nc.named_scope("attn1"):
                attn_block(1)
            out_partial(0)
            out_partial(1)
            with nc.named_scope("attn3"):
                attn_block(3)
            out_partial(2)
            out_partial(3)

            # ---- residual + LayerNorm on own rows ----
            def ln_chunk(ck):
                with nc.named_scope(f"ln{ck}"):
                    rs_sb = ln_pool.tile([128, D], BF, tag="rs")
                    nc.sync.dma_start(out=rs_sb[:, :], in_=rs_out[ck][:, :])
                    ev_t = ln_pool.tile([128, D], F32, tag="ev")
                    nc.sync.dma_start(out=ev_t[:, :], in_=ev_res[:, ck, :])
                    x_sb = ln_pool.tile([128, D], F32, tag="x")
                    nc.vector.tensor_add(x_sb[:, :], rs_sb[:, :], ev_t[:, :])
                    stats = small.tile([128, 2, 6], F32, tag="stats")
                    nc.vector.bn_stats(out=stats[:, 0, :], in_=x_sb[:, 0:512])
                    nc.vector.bn_stats(out=stats[:, 1, :], in_=x_sb[:, 512:1024])
                    mv = small.tile([128, 2], F32, tag="mv")
                    nc.vector.bn_aggr(out=mv[:, :], in_=stats[:, :, :])
                    std = small.tile([128, 1], F32, tag="std")
                    nc.scalar.activation(
                        std[:, :], mv[:, 1:2], AF.Sqrt, bias=eps_sb[:, :]
                    )
                    rstd = small.tile([128, 1], F32, tag="rstd")
                    nc.vector.reciprocal(rstd[:, :], std[:, :])
                    negmb = small.tile([128, 1], F32, tag="negmb")
                    nc.vector.scalar_tensor_tensor(
                        negmb[:, :], mv[:, 0:1], -1.0, rstd[:, :],
                        op0=ALU.mult, op1=ALU.mult,
                    )
                    nc.scalar.activation(
                        x_sb[:, :], x_sb[:, :], AF.Identity,
                        bias=negmb[:, :], scale=rstd[:, :],
                    )
                    nc.vector.tensor_mul(x_sb[:, :], x_sb[:, :], gb_bc[:, 0, :])
                    nc.vector.tensor_add(x_sb[:, :], x_sb[:, :], gb_bc[:, 1, :])
                    nc.sync.dma_start(
                        out=out[ck * 128:(ck + 1) * 128, :], in_=x_sb[:, :],
                    )

            for ck in range(4):
                ln_chunk(ck)

    nc.compile()
    return nc


def _get_nc():
    if "nc" not in _NC_CACHE:
        _NC_CACHE["nc"] = build_nc()
    return _NC_CACHE["nc"]


def make_in_maps(
    e_v, e_a0, e_a1, Wqv, bqv, Wkv, bkv, Wvv, bvv,
    Wqa0, bqa0, Wka0, bka0, Wqa1, bqa1, Wka1, bka1,
    Wout, bout, ln_gamma, ln_beta, attn_mask,
):
    f = np.asarray
    e_v, e_a0, e_a1 = f(e_v), f(e_a0), f(e_a1)
    attn_mask = f(attn_mask)

    def pmaj(a2d, dt, asdt=BF16):
        # [dt*128, X] -> [128, dt, X] partition-major
        a = np.asarray(a2d, dtype=np.float32)
        return np.ascontiguousarray(
            a.reshape(dt, 128, a.shape[1]).transpose(1, 0, 2).astype(asdt)
        )

    def wfold(w, S):
        # [D, C-slice] -> [NF, 128, NDT, 128] fold-major partition-major
        ws = np.asarray(w[:, S], dtype=np.float32)  # [D, 512]
        return np.ascontiguousarray(
            ws.reshape(NDT, 128, NF, 128).transpose(2, 1, 0, 3).astype(BF16)
        )

    cbf_row = lambda a: np.ascontiguousarray(
        np.asarray(a, dtype=np.float32).astype(BF16)
    ).reshape(1, -1)
    c32_row = lambda a: np.ascontiguousarray(
        np.asarray(a, dtype=np.float32)
    ).reshape(1, -1)

    wq_full = {"v": f(Wqv), "a0": f(Wqa0), "a1": f(Wqa1)}
    wk_full = {"v": f(Wkv), "a0": f(Wka0), "a1": f(Wka1)}
    bq_full = {"v": f(bqv), "a0": f(bqa0), "a1": f(bqa1)}
    bk_full = {"v": f(bkv), "a0": f(bka0), "a1": f(bka1)}

    xts = {b: {"v": pmaj(e_v[b].T, NDT), "a0": pmaj(e_a0[b].T, NDT),
               "a1": pmaj(e_a1[b].T, NDT)} for b in range(B)}
    # 0/1 diagonal template from the provided mask (causal block-Toeplitz)
    tri = np.ascontiguousarray(
        (attn_mask[0, 0, :128, :128].T == 0.0).astype(np.float32).astype(BF16)
    )

    in_maps = []
    for c in range(NCORES):
        b, hh = c // 2, c % 2
        S = slice(hh * C, (hh + 1) * C)
        m = {}
        for s in STREAMS:
            m[f"xt_{s}"] = xts[b][s]
            m[f"wq_{s}"] = wfold(wq_full[s], S)
            m[f"wk_{s}"] = wfold(wk_full[s], S)
            m[f"bq_{s}"] = cbf_row(bq_full[s][S])
            m[f"bk_{s}"] = cbf_row(bk_full[s][S])
        m["wv"] = pmaj(f(Wvv)[:, S], NDT)
        m["bv"] = cbf_row(f(bvv)[S])
        # wout rows = own ctx channels; [512, D] -> [128, NF, D]
        m["wout"] = pmaj(f(Wout)[S, :], NF)
        m["bout_half"] = cbf_row(f(bout) * 0.5)
        m["tri"] = tri
        ev = np.asarray(e_v[b, hh * 512:(hh + 1) * 512, :], dtype=np.float32)
        m["ev_res"] = np.ascontiguousarray(ev.reshape(4, 128, D).transpose(1, 0, 2))
        m["gamma"] = c32_row(f(ln_gamma))
        m["beta"] = c32_row(f(ln_beta))
        in_maps.append(m)
    return in_maps


def kernel(**inputs):
    global LAST_RESULT
    in_maps = make_in_maps(**inputs)
    nc = _get_nc()
    trace = bool(os.environ.get("KERNEL_TRACE"))
    res = run_bass_kernel_spmd(
        nc, in_maps, core_ids=list(range(NCORES)), trace=trace
    )
    LAST_RESULT = res

    out_full = np.empty((B, L, D), dtype=np.float32)
    for c in range(NCORES):
        b, hh = c // 2, c % 2
        out_full[b, hh * 512:(hh + 1) * 512, :] = res.results[c]["out"]
    return out_full


# revision 19
# speedup vs baseline: 1.0651x; 1.0651x over previous
"""DiffMHA (differential multi-head attention) block on 8 TRN2 NeuronCores.

Problem: B=4, L=1024, D=1024, H=16 heads (DH=64). Three input streams
(e_v, e_a0, e_a1); Q/K projections per stream, scores summed across
streams, causal-masked softmax, context from the v-stream values,
out-projection + residual + LayerNorm.

Sharding: (batch, head-half) -> 8 cores. Core c handles batch c//2 and
heads (c%2)*8 .. +8: its 8 heads' Q/K/V projections (512 of 1024
channels), scores + softmax + context for all 1024 rows, and a partial
out-projection. A pairwise bf16 ReduceScatter sums the two partial
out-projections and hands each core its own 512 rows for residual +
LayerNorm. The RS runs in 2 row-chunks so the first chunk's comm fully
overlaps the second half's attention compute; the PE never waits on a
collective.

Key optimizations vs the previous version:
- Stream-stacked score contraction: Q/K for streams (v, a0) are packed
  into one 128-partition tile per head at projection time (partition-
  shifted PSUM->SBUF copies), so scores take 2 matmuls (K=128 + K=64)
  instead of 3 K=64 matmuls -> fewer LDWEIGHTS, fuller PE array.
- Causal trimming: score/ctx matmuls only cover k-tiles with k <= q
  (q in 256-row blocks); the q-range is trimmed per k-tile. The mask
  input is replaced by one [128,128] 0/1 diagonal template applied
  multiplicatively after exp (exp(-10000/8) == 0 in f32 exactly).
- Head-pair fused score tiles: both heads of a fold share one
  [128, 512] PSUM score tile and a single Exp activation.
- Softmax 1/sum via DVE reciprocal_approx_fast (~5x faster than
  nc.vector.reciprocal) once per fold (both heads at once).
- bf16 chunked ReduceScatter (1 MB in per chunk vs one 4 MB fp32 RS),
  first chunk overlapped with attention blocks 1/3.
- All host-side tensors are partition-major so every DMA line is
  contiguous (>=2KB descriptors instead of 256B).
- All biases applied as rank-1 ones-row matmuls into PSUM.
- Load order puts xt_v + wv first so the PE starts within ~5us and the
  HAM clock-gate warms once and stays warm.
"""

import os
import sys
import types

import ml_dtypes
import numpy as np

B, L, D, H = 4, 1024, 1024, 16
DH = D // H  # 64
HPC = H // 2  # heads per core
C = HPC * DH  # channels per core (512)
SCALE = float(1.0 / np.sqrt(DH))
EPS = 1e-12
NCORES = 8
BF16 = ml_dtypes.bfloat16

NDT = D // 128  # 8 contraction folds
NF = C // 128  # 4 channel folds per core (2 heads each)
NLT = L // 128  # 8 l-tiles
QBLK = 256
STREAMS = ("v", "a0", "a1")
RG = [[0, 1], [2, 3], [4, 5], [6, 7]]


def _install_ntff_hook():
    """Recreate antenv.axon_hooks (absent in this image) so
    run_bass_kernel_spmd(trace=True) can capture NTFF profiles."""
    if "antenv.axon_hooks" in sys.modules:
        return
    try:
        from trn_agent_boot.trn_boot import _ntff_profile_via_ctypes

        hook = _ntff_profile_via_ctypes("/opt/axon/libaxon_pjrt.so")
    except Exception:
        hook = None
    mod = types.ModuleType("antenv.axon_hooks")
    mod.get_axon_ntff_profile_hook = lambda: hook
    mod.set_axon_ntff_profile_hook = lambda h: None
    sys.modules["antenv.axon_hooks"] = mod


_install_ntff_hook()

import concourse.bass as bass  # noqa: E402,F401
import concourse.mybir as mybir  # noqa: E402
import concourse.tile as tile  # noqa: E402
from concourse import bacc  # noqa: E402
from concourse.bass_utils import run_bass_kernel_spmd  # noqa: E402

F32 = mybir.dt.float32
BF = mybir.dt.bfloat16
AF = mybir.ActivationFunctionType
ALU = mybir.AluOpType

_NC_CACHE = {}
LAST_RESULT = None


def build_nc():
    nc = bacc.Bacc("TRN2", target_bir_lowering=False, debug=False, num_devices=NCORES)

    # ---- DRAM parameters (per-core shards, host-prepped, partition-major) ----
    xt = {s: nc.declare_dram_parameter(f"xt_{s}", [128, NDT, L], BF, isOutput=False)
          for s in STREAMS}
    wq = {s: nc.declare_dram_parameter(f"wq_{s}", [NF, 128, NDT, 128], BF, isOutput=False)
          for s in STREAMS}
    wk = {s: nc.declare_dram_parameter(f"wk_{s}", [NF, 128, NDT, 128], BF, isOutput=False)
          for s in STREAMS}
    bqr = {s: nc.declare_dram_parameter(f"bq_{s}", [1, C], BF, isOutput=False)
           for s in STREAMS}
    bkr = {s: nc.declare_dram_parameter(f"bk_{s}", [1, C], BF, isOutput=False)
           for s in STREAMS}
    wv = nc.declare_dram_parameter("wv", [128, NDT, C], BF, isOutput=False)
    bv = nc.declare_dram_parameter("bv", [1, C], BF, isOutput=False)
    wout = nc.declare_dram_parameter("wout", [128, NF, D], BF, isOutput=False)
    bout_half = nc.declare_dram_parameter("bout_half", [1, D], BF, isOutput=False)
    tri = nc.declare_dram_parameter("tri", [128, 128], BF, isOutput=False)
    ev_res = nc.declare_dram_parameter("ev_res", [128, 4, D], F32, isOutput=False)
    gamma = nc.declare_dram_parameter("gamma", [1, D], F32, isOutput=False)
    beta = nc.declare_dram_parameter("beta", [1, D], F32, isOutput=False)
    out = nc.declare_dram_parameter("out", [L // 2, D], F32, isOutput=True)

    with tile.TileContext(nc) as tc:
        with (
            tc.tile_pool(name="persist", bufs=1) as persist,
            tc.tile_pool(name="xtp", bufs=2) as xtp,
            tc.tile_pool(name="wf", bufs=4) as wf,
            tc.tile_pool(name="attn", bufs=3) as attn_pool,
            tc.tile_pool(name="small", bufs=2) as small,
            tc.tile_pool(name="cpy", bufs=3) as cpy_pool,
            tc.tile_pool(name="ln", bufs=2) as ln_pool,
            tc.tile_pool(name="proj_ps", bufs=2, space="PSUM") as proj_ps,
            tc.tile_pool(name="sc_ps", bufs=2, space="PSUM") as sc_ps,
            tc.tile_pool(name="ctx_ps", bufs=2, space="PSUM") as ctx_ps,
            tc.tile_pool(name="dram", bufs=1, space="DRAM") as dram,
        ):
            # ---- persistent SBUF ----
            # cols 0:63 = V, cols 64:127 = ones -> the ctx matmul lands
            # the softmax sum on PSUM partitions 64..127 (64 copies), so
            # no partition_broadcast is needed for the normalization.
            vnat = persist.tile([128, NLT, HPC, 2 * DH], BF, tag="vnat")
            ctxt = persist.tile([128, NF, L], BF, tag="ctxt")
            ones_b = persist.tile([1, L], BF, tag="ones")
            eps_sb = persist.tile([128, 1], F32, tag="eps")
            tri_sb = persist.tile([128, 128], BF, tag="tri")
            gb_bc = persist.tile([128, 2, D], F32, tag="gbbc")
            wv_sb = persist.tile([128, NDT, C], BF, tag="wvsb")
            wout_sb = persist.tile([128, NF, D], BF, tag="woutsb")
            bv_sb = persist.tile([1, C], BF, tag="bvsb")
            bout_sb = persist.tile([1, D], BF, tag="boutsb")
            bqr_sb = {s: persist.tile([1, C], BF, tag=f"bqr{s}", name=f"bqr_{s}")
                      for s in STREAMS}
            bkr_sb = {s: persist.tile([1, C], BF, tag=f"bkr{s}", name=f"bkr_{s}")
                      for s in STREAMS}
            # stacked Q/K tiles: per head h, partitions 0-63 = stream v
            # head-h channels, 64-127 = stream a0; a1 keeps fold tiles.
            qs1 = [persist.tile([128, L], BF, tag=f"qs1_{h}", name=f"qs1_{h}")
                   for h in range(HPC)]
            ks1 = [persist.tile([128, L], BF, tag=f"ks1_{h}", name=f"ks1_{h}")
                   for h in range(HPC)]
            qt2 = [persist.tile([128, L], BF, tag=f"qt2_{f}", name=f"qt2_{f}")
                   for f in range(NF)]
            kt2 = [persist.tile([128, L], BF, tag=f"kt2_{f}", name=f"kt2_{f}")
                   for f in range(NF)]

            # ---- first loads: unblock the PE ASAP (per-dt split so the
            # first V-proj matmuls start after ~256KB, not 3MB) ----
            xt_sb = {}
            t = xtp.tile([128, NDT, L], BF, tag="xt", name="xt_v")
            for dt in range(NDT):
                nc.sync.dma_start(out=t[:, dt, :], in_=xt["v"][:, dt, :])
                nc.sync.dma_start(out=wv_sb[:, dt, :], in_=wv[:, dt, :])
            xt_sb["v"] = t
            nc.sync.dma_start(out=bv_sb[:, :], in_=bv[:, :])
            nc.vector.memset(ones_b[:, :], 1.0)
            nc.vector.memset(eps_sb[:, :], EPS)
            nc.vector.memset(vnat[:, :, :, DH:2 * DH], 1.0)

            # HAM warm-up: ~3.5us of dummy matmuls (no DMA dependency) while
            # the first input DMAs land, so real matmuls start at 2.4GHz.
            warm = proj_ps.tile([128, 512], F32, tag="proj")
            for i in range(48):
                nc.tensor.matmul(
                    warm[:, 0:256], ones_b[:, 0:128], ones_b[:, 0:256],
                    start=(i == 0), stop=(i == 47),
                )

            # ---- V projection (natural [l, c] layout + ones column) ----
            with nc.named_scope("vproj"):
                for lf in range(NLT):
                    ps = proj_ps.tile([128, 512], F32, tag="proj")
                    for dt in range(NDT):
                        nc.tensor.matmul(
                            ps[:, :],
                            xt_sb["v"][:, dt, lf * 128:(lf + 1) * 128],
                            wv_sb[:, dt, :],
                            start=(dt == 0), stop=False,
                        )
                    nc.tensor.matmul(
                        ps[:, :], ones_b[:, 0:128], bv_sb[:, :],
                        start=False, stop=True,
                    )
                    nc.scalar.copy(vnat[:, lf, :, 0:DH], ps[:, :])

            # ---- rest of the loads (overlap with V proj) ----
            t = xtp.tile([128, NDT, L], BF, tag="xt", name="xt_a0")
            nc.sync.dma_start(out=t[:, :, :], in_=xt["a0"][:, :, :])
            xt_sb["a0"] = t
            nc.sync.dma_start(out=tri_sb[:, :], in_=tri[:, :])
            for s in STREAMS:
                nc.sync.dma_start(out=bqr_sb[s][:, :], in_=bqr[s][:, :])
                nc.sync.dma_start(out=bkr_sb[s][:, :], in_=bkr[s][:, :])
            nc.sync.dma_start(out=bout_sb[:, :], in_=bout_half[:, :])
            nc.sync.dma_start(out=wout_sb[:, :, :], in_=wout[:, :, :])
            gsb = small.tile([1, D], F32, tag="gsb", bufs=1)
            bsb = small.tile([1, D], F32, tag="bsb", bufs=1)
            nc.sync.dma_start(out=gsb[:, :], in_=gamma[:, :])
            nc.sync.dma_start(out=bsb[:, :], in_=beta[:, :])
            nc.gpsimd.partition_broadcast(gb_bc[:, 0, :], gsb[:, :])
            nc.gpsimd.partition_broadcast(gb_bc[:, 1, :], bsb[:, :])

            # ---- Q/K projections ----
            # v/a0 results are split per head into the stacked qs1/ks1
            # tiles via partition-shifted copies; a1 keeps fold tiles.
            def qk_proj(s, f, which):
                w_dram = wq[s] if which == "q" else wk[s]
                brow = bqr_sb[s] if which == "q" else bkr_sb[s]
                dst1 = qs1 if which == "q" else ks1
                dst2 = qt2 if which == "q" else kt2
                w_t = wf.tile([128, NDT, 128], BF, tag="w", name=f"w{which}_{s}{f}")
                nc.sync.dma_start(out=w_t[:, :, :], in_=w_dram[f, :, :, :])
                for lh in range(2):
                    lsl = slice(lh * 512, (lh + 1) * 512)
                    ps = proj_ps.tile([128, 512], F32, tag="proj")
                    for dt in range(NDT):
                        nc.tensor.matmul(
                            ps[:, :], w_t[:, dt, :], xt_sb[s][:, dt, lsl],
                            start=(dt == 0), stop=False,
                        )
                    nc.tensor.matmul(
                        ps[:, :], brow[:, f * 128:(f + 1) * 128],
                        ones_b[:, 0:512], start=False, stop=True,
                    )
                    if s == "a1":
                        nc.scalar.copy(dst2[f][:, lsl], ps[:, :])
                    elif s == "v":
                        nc.scalar.copy(dst1[2 * f][0:64, lsl], ps[0:64, :])
                        nc.scalar.copy(dst1[2 * f + 1][0:64, lsl], ps[64:128, :])
                    else:  # a0
                        nc.scalar.copy(dst1[2 * f][64:128, lsl], ps[0:64, :])
                        nc.scalar.copy(dst1[2 * f + 1][64:128, lsl], ps[64:128, :])

            with nc.named_scope("qkproj_v_a0"):
                for s in ("v", "a0"):
                    for f in range(NF):
                        qk_proj(s, f, "q")
                        qk_proj(s, f, "k")

            # xt_a1 reuses xt_v's slot (xtp bufs=2); the DMA waits for the
            # last v-stream projection read automatically.
            t = xtp.tile([128, NDT, L], BF, tag="xt", name="xt_a1")
            nc.sync.dma_start(out=t[:, :, :], in_=xt["a1"][:, :, :])
            xt_sb["a1"] = t

            # ---- attention for one q-block of 256 rows ----
            def attn_block(b):
                q0 = b * QBLK
                for f in range(NF):
                    hA, hB = 2 * f, 2 * f + 1
                    cps = ctx_ps.tile([2 * DH, 2 * QBLK], F32, tag="ctx")
                    last_kt = 2 * b + 1

                    def ctx_mm(attn_sb, kt, trim, n):
                        nc.tensor.matmul(
                            cps[:, trim:QBLK], vnat[:, kt, hA, :],
                            attn_sb[:, 0:n],
                            start=(kt == 0), stop=False,
                        )
                        nc.tensor.matmul(
                            cps[:, QBLK + trim:2 * QBLK], vnat[:, kt, hB, :],
                            attn_sb[:, 256:256 + n],
                            start=False, stop=(kt == last_kt),
                        )

                    # software pipeline: emit kt's ctx matmuls after kt+1's
                    # score matmuls, so the Exp latency never blocks the
                    # in-order PE queue.
                    pend = None
                    for kt in range(last_kt + 1):
                        trim = 128 if kt == last_kt else 0
                        n = QBLK - trim
                        qsl = slice(q0 + trim, q0 + QBLK)
                        ksl = slice(kt * 128, (kt + 1) * 128)
                        # score regions in SEPARATE PSUM banks of one tile
                        # (A at cols 0.., B at 512..): full-row matmuls
                        # first so LDWEIGHTS overlaps via the background
                        # weight buffer; the 64-row a1 matmuls may then run
                        # concurrently (disjoint rows) and legally drain
                        # into DIFFERENT banks. Same-bank concurrent drains
                        # crash the exec unit.
                        sps = sc_ps.tile([128, 4 * QBLK], F32, tag="sc")
                        rA, rB = slice(0, n), slice(512, 512 + n)
                        nc.tensor.matmul(
                            sps[:, rA], ks1[hA][:, ksl], qs1[hA][:, qsl],
                            start=True, stop=False,
                        )
                        nc.tensor.matmul(
                            sps[:, rB], ks1[hB][:, ksl], qs1[hB][:, qsl],
                            start=True, stop=False,
                        )
                        nc.tensor.matmul(
                            sps[:, rA], kt2[f][0:64, ksl],
                            qt2[f][0:64, qsl], start=False, stop=True,
                        )
                        nc.tensor.matmul(
                            sps[:, rB], kt2[f][64:128, ksl],
                            qt2[f][64:128, qsl], start=False, stop=True,
                        )
                        if pend is not None:
                            ctx_mm(*pend)
                        # one Exp over both regions via a strided dual-bank
                        # access pattern (bank-crossing is matmul-only).
                        attn_sb = attn_pool.tile([128, 2 * QBLK], BF, tag="attn")
                        sps_v = sps[:, :].rearrange("p (c x) -> p c x", x=512)
                        attn_v = attn_sb[:, :].rearrange("p (c x) -> p c x", x=256)
                        nc.scalar.activation(
                            attn_v[:, :, 0:n], sps_v[:, :, 0:n], AF.Exp, scale=SCALE
                        )
                        if kt >= 2 * b:  # diagonal k-tile: 0/1 causal mask
                            nc.vector.tensor_mul(
                                attn_sb[:, 0:128], attn_sb[:, 0:128], tri_sb[:, :]
                            )
                            nc.vector.tensor_mul(
                                attn_sb[:, 256:384], attn_sb[:, 256:384],
                                tri_sb[:, :],
                            )
                        pend = (attn_sb, kt, trim, n)
                    ctx_mm(*pend)
                    # normalize both heads: ctx /= sum. PSUM rows 64:128
                    # hold 64 copies of the sum (ones-columns trick). Copy
                    # to SBUF first: custom-DVE ops mis-execute on HW when
                    # reading PSUM directly.
                    ssb = small.tile([64, 2 * QBLK], F32, tag="ssb")
                    nc.vector.tensor_copy(ssb[:, :], cps[DH:2 * DH, :])
                    invbc = small.tile([64, 2 * QBLK], F32, tag="invbc")
                    nc.vector.reciprocal_approx_fast(invbc[:, :], ssb[:, :])
                    nc.vector.tensor_mul(
                        ctxt[0:64, f, q0:q0 + QBLK],
                        cps[0:DH, 0:QBLK], invbc[:, 0:QBLK],
                    )
                    nc.vector.tensor_mul(
                        ctxt[64:128, f, q0:q0 + QBLK],
                        cps[0:DH, QBLK:2 * QBLK], invbc[:, QBLK:2 * QBLK],
                    )

            # a1 projections fold-major, interleaved with attention on the
            # first q-block so the PE pipeline never drains.
            with nc.named_scope("qkproj_a1_attn0"):
                for f in range(NF):
                    qk_proj("a1", f, "q")
                    qk_proj("a1", f, "k")
                attn_block(0)
            with nc.named_scope("attn2"):
                attn_block(2)

            # ---- partial out-projection + chunked bf16 ReduceScatter ----
            # Chunk ck covers global rows {j*512 + ck*256 ..+256, j=0,1};
            # the RS hands each core the summed rows it owns (rank order
            # == chunk order makes this core-uniform).
            rs_in = [dram.tile([2, QBLK, D], BF, name=f"rs_in{k}") for k in range(2)]
            rs_out = [dram.tile([QBLK, D], BF, name=f"rs_out{k}") for k in range(2)]

            def out_partial(ck):
                with nc.named_scope(f"outpart{ck}"):
                    for j in range(2):
                      for lt in range(2):
                        gq0 = j * 512 + ck * QBLK + lt * 128
                        for half in range(2):
                            dsl = slice(half * 512, (half + 1) * 512)
                            ps = proj_ps.tile([128, 512], F32, tag="proj")
                            for cf in range(NF):
                                nc.tensor.matmul(
                                    ps[:, :], ctxt[:, cf, gq0:gq0 + 128],
                                    wout_sb[:, cf, dsl],
                                    start=(cf == 0), stop=False,
                                )
                            nc.tensor.matmul(
                                ps[:, :], ones_b[:, 0:128], bout_sb[:, dsl],
                                start=False, stop=True,
                            )
                            pt = cpy_pool.tile([128, 512], BF, tag="pt")
                            nc.vector.tensor_copy(pt[:, :], ps[:, :])
                            nc.sync.dma_start(
                                out=rs_in[ck][j, lt * 128:(lt + 1) * 128, dsl],
                                in_=pt[:, :],
                            )
                    nc.gpsimd.collective_compute(
                        "ReduceScatter", ALU.add, replica_groups=RG,
                        ins=[rs_in[ck].opt()], outs=[rs_out[ck].opt()],
                    )

            # block 1 before the first out_partials: the in-order PE queue
            # must not park out-proj matmuls (waiting on block 2's
            # normalization) ahead of ready attention work.
            with nc.named_scope("attn1"):
                attn_block(1)
            out_partial(0)
            with nc.named_scope("attn3"):
                attn_block(3)
            out_partial(1)

            # ---- residual + LayerNorm on own rows ----
            def ln_chunk(ck):
                with nc.named_scope(f"ln{ck}"):
                  for lt in range(2):
                    lti = ck * 2 + lt
                    rs_sb = ln_pool.tile([128, D], BF, tag="rs")
                    nc.sync.dma_start(
                        out=rs_sb[:, :],
                        in_=rs_out[ck][lt * 128:(lt + 1) * 128, :])
                    ev_t = ln_pool.tile([128, D], F32, tag="ev")
                    nc.sync.dma_start(out=ev_t[:, :], in_=ev_res[:, lti, :])
                    x_sb = ln_pool.tile([128, D], F32, tag="x")
                    nc.vector.tensor_add(x_sb[:, :], rs_sb[:, :], ev_t[:, :])
                    stats = small.tile([128, 2, 6], F32, tag="stats")
                    nc.vector.bn_stats(out=stats[:, 0, :], in_=x_sb[:, 0:512])
                    nc.vector.bn_stats(out=stats[:, 1, :], in_=x_sb[:, 512:1024])
                    mv = small.tile([128, 2], F32, tag="mv")
                    nc.vector.bn_aggr(out=mv[:, :], in_=stats[:, :, :])
                    std = small.tile([128, 1], F32, tag="std")
                    nc.scalar.activation(
                        std[:, :], mv[:, 1:2], AF.Sqrt, bias=eps_sb[:, :]
                    )
                    rstd = small.tile([128, 1], F32, tag="rstd")
                    nc.vector.reciprocal(rstd[:, :], std[:, :])
                    negmb = small.tile([128, 1], F32, tag="negmb")
                    nc.vector.scalar_tensor_tensor(
                        negmb[:, :], mv[:, 0:1], -1.0, rstd[:, :],
                        op0=ALU.mult, op1=ALU.mult,
                    )
                    nc.scalar.activation(
                        x_sb[:, :], x_sb[:, :], AF.Identity,
                        bias=negmb[:, :], scale=rstd[:, :],
                    )
                    nc.vector.tensor_mul(x_sb[:, :], x_sb[:, :], gb_bc[:, 0, :])
                    nc.vector.tensor_add(x_sb[:, :], x_sb[:, :], gb_bc[:, 1, :])
                    nc.sync.dma_start(
                        out=out[lti * 128:(lti + 1) * 128, :], in_=x_sb[:, :],
                    )

            ln_chunk(0)
            ln_chunk(1)

    nc.compile()
    return nc


def _get_nc():
    if "nc" not in _NC_CACHE:
        _NC_CACHE["nc"] = build_nc()
    return _NC_CACHE["nc"]


def make_in_maps(
    e_v, e_a0, e_a1, Wqv, bqv, Wkv, bkv, Wvv, bvv,
    Wqa0, bqa0, Wka0, bka0, Wqa1, bqa1, Wka1, bka1,
    Wout, bout, ln_gamma, ln_beta, attn_mask,
):
    f = np.asarray
    e_v, e_a0, e_a1 = f(e_v), f(e_a0), f(e_a1)
    attn_mask = f(attn_mask)

    def pmaj(a2d, dt, asdt=BF16):
        # [dt*128, X] -> [128, dt, X] partition-major
        a = np.asarray(a2d, dtype=np.float32)
        return np.ascontiguousarray(
            a.reshape(dt, 128, a.shape[1]).transpose(1, 0, 2).astype(asdt)
        )

    def wfold(w, S):
        # [D, C-slice] -> [NF, 128, NDT, 128] fold-major partition-major
        ws = np.asarray(w[:, S], dtype=np.float32)  # [D, 512]
        return np.ascontiguousarray(
            ws.reshape(NDT, 128, NF, 128).transpose(2, 1, 0, 3).astype(BF16)
        )

    cbf_row = lambda a: np.ascontiguousarray(
        np.asarray(a, dtype=np.float32).astype(BF16)
    ).reshape(1, -1)
    c32_row = lambda a: np.ascontiguousarray(
        np.asarray(a, dtype=np.float32)
    ).reshape(1, -1)

    wq_full = {"v": f(Wqv), "a0": f(Wqa0), "a1": f(Wqa1)}
    wk_full = {"v": f(Wkv), "a0": f(Wka0), "a1": f(Wka1)}
    bq_full = {"v": f(bqv), "a0": f(bqa0), "a1": f(bqa1)}
    bk_full = {"v": f(bkv), "a0": f(bka0), "a1": f(bka1)}

    xts = {b: {"v": pmaj(e_v[b].T, NDT), "a0": pmaj(e_a0[b].T, NDT),
               "a1": pmaj(e_a1[b].T, NDT)} for b in range(B)}
    # 0/1 diagonal template from the provided mask (causal block-Toeplitz)
    tri = np.ascontiguousarray(
        (attn_mask[0, 0, :128, :128].T == 0.0).astype(np.float32).astype(BF16)
    )

    in_maps = []
    for c in range(NCORES):
        b, hh = c // 2, c % 2
        S = slice(hh * C, (hh + 1) * C)
        m = {}
        for s in STREAMS:
            m[f"xt_{s}"] = xts[b][s]
            m[f"wq_{s}"] = wfold(wq_full[s], S)
            m[f"wk_{s}"] = wfold(wk_full[s], S)
            m[f"bq_{s}"] = cbf_row(bq_full[s][S])
            m[f"bk_{s}"] = cbf_row(bk_full[s][S])
        m["wv"] = pmaj(f(Wvv)[:, S], NDT)
        m["bv"] = cbf_row(f(bvv)[S])
        # wout rows = own ctx channels; [512, D] -> [128, NF, D]
        m["wout"] = pmaj(f(Wout)[S, :], NF)
        m["bout_half"] = cbf_row(f(bout) * 0.5)
        m["tri"] = tri
        ev = np.asarray(e_v[b, hh * 512:(hh + 1) * 512, :], dtype=np.float32)
        m["ev_res"] = np.ascontiguousarray(ev.reshape(4, 128, D).transpose(1, 0, 2))
        m["gamma"] = c32_row(f(ln_gamma))
        m["beta"] = c32_row(f(ln_beta))
        in_maps.append(m)
    return in_maps


def kernel(**inputs):
    global LAST_RESULT
    in_maps = make_in_maps(**inputs)
    nc = _get_nc()
    trace = bool(os.environ.get("KERNEL_TRACE"))
    res = run_bass_kernel_spmd(
        nc, in_maps, core_ids=list(range(NCORES)), trace=trace
    )
    LAST_RESULT = res

    out_full = np.empty((B, L, D), dtype=np.float32)
    for c in range(NCORES):
        b, hh = c // 2, c % 2
        out_full[b, hh * 512:(hh + 1) * 512, :] = res.results[c]["out"]
    return out_full
